# revision 6
# baseline (speedup 1.0000x reference)
"""AttentionUpscaling Trainium2 kernel.

Device (8 NeuronCores, pure data-parallel over batch): per core one batch's
rec = attn (1024x1024) @ hf (1024x3072) on the TensorEngine, fp8(e3m4)
operands holding int4 quantized values, fp32 PSUM accumulation.

The device invocation is transfer-bound over the axon tunnel, so everything
ships int4-packed as uint8 nibbles:
  in  X [1024, 2048] u8: cols 0..511  = attnT nibbles (unsigned, cols j / j+512)
                         cols 512..2047 = hf nibbles (offset-8, cols j / j+1536)
  out Y [1024, 1540] u8: cols 0..1535 = rec nibbles (offset-8, cols j / j+1536)
                         cols 1536..1539 = per-row f32 |rec|max (device units)
Device unpacks nibbles to fp8 (exact small ints), matmuls, row-max-quantizes
the result back to int4. Host: gaussian blur / high-frequency extraction,
unfold/fold, bicubic base upsample, quant/dequant, final add.
"""

import os
import sys

import numpy as np

sys.path.insert(0, "/opt/trn_rl_repo")

B, C, HR, LRS = 8, 3, 1024, 256
P = 32          # HR patch size (KERNEL_SIZE=8 * scale=4)
N = 1024        # number of patches = (1024/32)**2
D = 3072        # C * P * P
BLUR_KS = 7
BLUR_SIGMA = 1.5
N_CORES = 8

HF_CLIP_SIGMA = 3.2     # hf int4 grid covers +-3.2 sigma

_CACHE = {}
LAST_RESULTS = None


# ----------------------------------------------------------------- host math
def _gauss1d(ks, sigma):
    c = np.arange(ks, dtype=np.float32) - (ks - 1) / 2.0
    g = np.exp(-(c * c) / (2.0 * sigma * sigma))
    return (g / g.sum()).astype(np.float32)


def _blur(x):
    # depthwise separable 7-tap gaussian, reflect padding (matches reference)
    g = _gauss1d(BLUR_KS, BLUR_SIGMA)
    pad = BLUR_KS // 2
    tmp = np.empty_like(x)
    xp = np.pad(x, ((0, 0), (0, 0), (pad, pad), (0, 0)), mode="reflect")
    acc = np.zeros_like(x)
    for k in range(BLUR_KS):
        np.multiply(xp[:, :, k : k + x.shape[2], :], g[k], out=tmp)
        np.add(acc, tmp, out=acc)
    xp = np.pad(acc, ((0, 0), (0, 0), (0, 0), (pad, pad)), mode="reflect")
    acc.fill(0.0)
    for k in range(BLUR_KS):
        np.multiply(xp[:, :, :, k : k + x.shape[3]], g[k], out=tmp)
        np.add(acc, tmp, out=acc)
    return acc


def _keys_cubic(x):
    # jax.image.resize 'bicubic' kernel (Keys, a = -0.5)
    x = np.abs(x)
    out = np.where(x <= 1.0, (1.5 * x - 2.5) * x * x + 1.0, 0.0)
    out = np.where(
        (x > 1.0) & (x < 2.0), ((-0.5 * x + 2.5) * x - 4.0) * x + 2.0, out
    )
    return out.astype(np.float32)


def _resize_weight_mat(in_size, out_size):
    # port of jax.image compute_weight_mat (antialias upscale -> kernel_scale 1)
    inv_scale = in_size / out_size
    sample_f = (np.arange(out_size, dtype=np.float64) + 0.5) * inv_scale - 0.5
    x = np.abs(sample_f[None, :] - np.arange(in_size, dtype=np.float64)[:, None])
    w = _keys_cubic(x).astype(np.float64)
    total = w.sum(axis=0, keepdims=True)
    w = np.where(np.abs(total) > 1000.0 * np.finfo(np.float32).eps, w / total, 0.0)
    w = np.where(
        ((sample_f >= -0.5) & (sample_f <= in_size - 0.5))[None, :], w, 0.0
    )
    return w.astype(np.float32)  # (in_size, out_size)


def _bicubic_base(x_lr):
    w = _resize_weight_mat(LRS, HR)  # (256, 1024)
    flat = x_lr.reshape(B * C, LRS, LRS)
    t = np.matmul(w.T[None].astype(np.float32), flat)       # (BC, 1024, 256)
    out = np.matmul(t, w[None].astype(np.float32))          # (BC, 1024, 1024)
    return out.reshape(B, C, HR, HR)


# ------------------------------------------------------------- device kernel
def _build_bass():
    import concourse.bacc as bacc
    import concourse.mybir as mybir
    from concourse.tile import TileContext
    from concourse.alu_op_type import AluOpType

    nc = bacc.Bacc(None, target_bir_lowering=False)
    x = nc.dram_tensor("x", [N, 2048], mybir.dt.uint8, kind="ExternalInput")
    y = nc.dram_tensor("y", [N, 1540], mybir.dt.uint8, kind="ExternalOutput")

    KT = N // 128   # 8 contraction tiles
    NT = N // 128   # 8 output-row tiles
    ND = D // 512   # 6 psum tiles per output-row tile

    f8 = mybir.dt.float8e3
    f32 = mybir.dt.float32
    u8 = mybir.dt.uint8

    with TileContext(nc) as tc:
        with (
            tc.tile_pool(name="xp", bufs=1) as xp,
            tc.tile_pool(name="qp", bufs=2) as qp,
            tc.tile_pool(name="otp", bufs=2) as otp,
            tc.tile_pool(name="psp", bufs=1, space="PSUM") as psp,
        ):
            at_sb, hf_sb = [], []
            for k in range(KT):
                xt = xp.tile([128, 2048], u8, name=f"x{k}")
                nc.sync.dma_start(xt[:], x[k * 128 : (k + 1) * 128, :])
                at = xp.tile([128, N], f8, name=f"a{k}")
                ht = xp.tile([128, D], f8, name=f"h{k}")
                # walrus rejects fusing bitwise with arith ops (and, to be
                # safe, dtype-converting writes from bitwise ops), so unpack
                # is: pure-bitwise u8->u8, then an arith convert to fp8.
                ta = qp.tile([128, 512], u8, name="ta", tag="ta")
                tb = qp.tile([128, 512], u8, name="tb", tag="tb")
                tl = qp.tile([128, 1536], u8, name="tl", tag="tl")
                th = qp.tile([128, 1536], u8, name="th", tag="th")
                # attn nibbles: unsigned 0..15
                nc.vector.tensor_scalar(
                    ta[:], xt[:, 0:512], 15, None, AluOpType.bitwise_and
                )
                nc.vector.tensor_copy(at[:, 0:512], ta[:])
                nc.vector.tensor_scalar(
                    tb[:], xt[:, 0:512], 4, None, AluOpType.logical_shift_right
                )
                nc.vector.tensor_copy(at[:, 512:1024], tb[:])
                # hf nibbles: offset-8 signed
                nc.vector.tensor_scalar(
                    tl[:], xt[:, 512:2048], 15, None, AluOpType.bitwise_and
                )
                nc.vector.tensor_scalar_sub(ht[:, 0:1536], tl[:], 8)
                nc.vector.tensor_scalar(
                    th[:], xt[:, 512:2048], 4, None,
                    AluOpType.logical_shift_right,
                )
                nc.vector.tensor_scalar_sub(ht[:, 1536:3072], th[:], 8)
                at_sb.append(at)
                hf_sb.append(ht)

            for n in range(NT):
                ncols = slice(n * 128, (n + 1) * 128)
                ps = [
                    psp.tile([128, 512], f32, name=f"ps{d}", tag=f"ps{d}")
                    for d in range(ND)
                ]
                for k in range(KT):
                    for d in range(ND):
                        nc.tensor.matmul(
                            ps[d][:],
                            at_sb[k][:, ncols],
                            hf_sb[k][:, d * 512 : (d + 1) * 512],
                            start=(k == 0),
                            stop=(k == KT - 1),
                        )
                # per-row |max| over all 6 tiles -> quant scale
                m6 = qp.tile([128, ND], f32, name="m6", tag="m6")
                for d in range(ND):
                    nc.vector.tensor_reduce(
                        m6[:, d : d + 1], ps[d][:], mybir.AxisListType.X,
                        AluOpType.max, apply_absolute_value=True,
                    )
                M = qp.tile([128, 1], f32, name="M", tag="M")
                nc.vector.tensor_reduce(
                    M[:], m6[:], mybir.AxisListType.X, AluOpType.max
                )
                nc.vector.tensor_scalar_max(M[:], M[:], 1e-20)
                S = qp.tile([128, 1], f32, name="S", tag="S")
                nc.vector.reciprocal(S[:], M[:])
                nc.vector.tensor_scalar_mul(S[:], S[:], 7.49)

                qt = [
                    qp.tile([128, 512], u8, name=f"q{d}", tag=f"q{d}")
                    for d in range(ND)
                ]
                t1 = [
                    qp.tile([128, 512], f32, name=f"t{d}", tag=f"t{d}")
                    for d in range(ND)
                ]
                for d in range(ND):
                    # HW f32->u8 conversion rounds-to-nearest (CoreSim
                    # truncates!), so offset by 8.0: u8(clamp(ps*S + 8))
                    # = round(ps*S) + 8 on hardware.
                    nc.vector.tensor_scalar(
                        t1[d][:], ps[d][:], S[:], 8.0,
                        AluOpType.mult, AluOpType.add,
                    )
                    nc.vector.tensor_scalar(
                        qt[d][:], t1[d][:], 0.0, 15.49,
                        AluOpType.max, AluOpType.min,
                    )
                ot = otp.tile([128, 1540], u8, name="ot", tag="ot")
                for d in range(3):
                    nc.vector.scalar_tensor_tensor(
                        ot[:, d * 512 : (d + 1) * 512],
                        qt[d + 3][:], 16.0, qt[d][:],
                        AluOpType.mult, AluOpType.add,
                    )
                nc.vector.tensor_copy(
                    ot[:, 1536:1540], M[:].bitcast(u8)
                )
                nc.gpsimd.dma_start(y[n * 128 : (n + 1) * 128, :], ot[:])
    nc.compile()
    return nc


def _get_nc():
    if "nc" not in _CACHE:
        _CACHE["nc"] = _build_bass()
    return _CACHE["nc"]


# ---------------------------------------------------------------- entrypoint
def kernel(x_hr, x_lr_inpainted, attn_map):
    global LAST_RESULTS
    from concourse import bass_utils

    x_hr = np.asarray(x_hr, dtype=np.float32)
    x_lr = np.asarray(x_lr_inpainted, dtype=np.float32)
    attn = np.asarray(attn_map, dtype=np.float32)

    # high-frequency residual -> patch layout [m=(i,j), d=(c,ph,pw)]
    hp = x_hr - _blur(x_hr)
    hfm = (
        hp.reshape(B, C, HR // P, P, HR // P, P)
        .transpose(0, 2, 4, 1, 3, 5)
        .reshape(B, N, D)
    )
    attnT = np.ascontiguousarray(attn[:, 0].transpose(0, 2, 1))

    # int4 quantization (per batch element = per core)
    sa = 15.0 / attnT.reshape(B, -1).max(axis=1)            # (B,)
    sh = 7.49 / (HF_CLIP_SIGMA * hfm.reshape(B, -1).std(axis=1))
    qa = np.clip(
        np.round(attnT * sa[:, None, None]), 0, 15
    ).astype(np.uint8)
    qh = (
        np.clip(np.round(hfm * sh[:, None, None]), -8, 7) + 8
    ).astype(np.uint8)

    xin = np.empty((B, N, 2048), dtype=np.uint8)
    xin[:, :, 0:512] = qa[:, :, 0:512] + (qa[:, :, 512:1024] << 4)
    xin[:, :, 512:2048] = qh[:, :, 0:1536] + (qh[:, :, 1536:3072] << 4)

    nc = _get_nc()
    if not os.environ.get("KERNEL_TRACE"):
        # NTFF profiling hook (antenv.axon_hooks) is absent in this
        # container; a stray BASS_TRACE=1 would crash the run.
        os.environ["BASS_NEVER_TRACE"] = "1"
    in_maps = [{"x": xin[b]} for b in range(N_CORES)]
    res = bass_utils.run_bass_kernel_spmd(
        nc, in_maps, core_ids=list(range(N_CORES)),
        trace=bool(os.environ.get("KERNEL_TRACE")),
    )
    LAST_RESULTS = res
    _CACHE["in_maps"] = in_maps

    yout = np.stack([np.asarray(res.results[b]["y"]) for b in range(N_CORES)])
    rowmax = yout[:, :, 1536:1540].copy().view(np.float32)[:, :, 0]  # (B, N)
    step = rowmax / 7.49                                    # device units
    lo = (yout[:, :, 0:1536] & 15).astype(np.float32) - 8.0
    hi = (yout[:, :, 0:1536] >> 4).astype(np.float32) - 8.0
    rec = np.empty((B, N, D), dtype=np.float32)
    rec[:, :, 0:1536] = lo
    rec[:, :, 1536:3072] = hi
    rec *= (step / (sa * sh)[:, None])[:, :, None]

    rec_img = (
        rec.reshape(B, HR // P, HR // P, C, P, P)
        .transpose(0, 3, 1, 4, 2, 5)
        .reshape(B, C, HR, HR)
    )
    base = _bicubic_base(x_lr)
    return (base + rec_img).astype(np.float32)


def time_device(n=5):
    """Best-of-n wall time of the device invocation (post-compile)."""
    import time as _time

    from concourse import bass_utils

    nc = _get_nc()
    in_maps = _CACHE["in_maps"]
    best = float("inf")
    for _ in range(n):
        t0 = _time.time()
        bass_utils.run_bass_kernel_spmd(
            nc, in_maps, core_ids=list(range(N_CORES))
        )
        best = min(best, _time.time() - t0)
    return best


# revision 7
# speedup vs baseline: 1.1082x; 1.1082x over previous
"""AttentionUpscaling Trainium2 kernel.

Device (8 NeuronCores, pure data-parallel over batch): per core one batch's
rec = attn (1024x1024) @ hf (1024x3072) on the TensorEngine, fp8(e3m4)
operands holding int4 quantized values, fp32 PSUM accumulation.

The device invocation is transfer-bound over the axon tunnel, so everything
ships int4-packed as uint8 nibbles:
  in  X [1024, 2048] u8: cols 0..511  = attnT nibbles (unsigned, cols j / j+512)
                         cols 512..2047 = hf nibbles (offset-8, cols j / j+1536)
  out Y [1024, 1540] u8: cols 0..1535 = rec nibbles (offset-8, cols j / j+1536)
                         cols 1536..1539 = per-row f32 |rec|max (device units)
Device unpacks nibbles to fp8 (exact small ints), matmuls, row-max-quantizes
the result back to int4. Host: gaussian blur / high-frequency extraction,
unfold/fold, bicubic base upsample, quant/dequant, final add.
"""

import os
import sys

import numpy as np

sys.path.insert(0, "/opt/trn_rl_repo")

B, C, HR, LRS = 8, 3, 1024, 256
P = 32          # HR patch size (KERNEL_SIZE=8 * scale=4)
N = 1024        # number of patches = (1024/32)**2
D = 3072        # C * P * P
BLUR_KS = 7
BLUR_SIGMA = 1.5
N_CORES = 8

HF_CLIP_SIGMA = 3.2     # hf int4 grid covers +-3.2 sigma

_CACHE = {}
LAST_RESULTS = None


# ----------------------------------------------------------------- host math
def _gauss1d(ks, sigma):
    c = np.arange(ks, dtype=np.float32) - (ks - 1) / 2.0
    g = np.exp(-(c * c) / (2.0 * sigma * sigma))
    return (g / g.sum()).astype(np.float32)


def _blur(x):
    # depthwise separable 7-tap gaussian, reflect padding (matches reference)
    g = _gauss1d(BLUR_KS, BLUR_SIGMA)
    pad = BLUR_KS // 2
    tmp = np.empty_like(x)
    xp = np.pad(x, ((0, 0), (0, 0), (pad, pad), (0, 0)), mode="reflect")
    acc = np.zeros_like(x)
    for k in range(BLUR_KS):
        np.multiply(xp[:, :, k : k + x.shape[2], :], g[k], out=tmp)
        np.add(acc, tmp, out=acc)
    xp = np.pad(acc, ((0, 0), (0, 0), (0, 0), (pad, pad)), mode="reflect")
    acc.fill(0.0)
    for k in range(BLUR_KS):
        np.multiply(xp[:, :, :, k : k + x.shape[3]], g[k], out=tmp)
        np.add(acc, tmp, out=acc)
    return acc


def _keys_cubic(x):
    # jax.image.resize 'bicubic' kernel (Keys, a = -0.5)
    x = np.abs(x)
    out = np.where(x <= 1.0, (1.5 * x - 2.5) * x * x + 1.0, 0.0)
    out = np.where(
        (x > 1.0) & (x < 2.0), ((-0.5 * x + 2.5) * x - 4.0) * x + 2.0, out
    )
    return out.astype(np.float32)


def _resize_weight_mat(in_size, out_size):
    # port of jax.image compute_weight_mat (antialias upscale -> kernel_scale 1)
    inv_scale = in_size / out_size
    sample_f = (np.arange(out_size, dtype=np.float64) + 0.5) * inv_scale - 0.5
    x = np.abs(sample_f[None, :] - np.arange(in_size, dtype=np.float64)[:, None])
    w = _keys_cubic(x).astype(np.float64)
    total = w.sum(axis=0, keepdims=True)
    w = np.where(np.abs(total) > 1000.0 * np.finfo(np.float32).eps, w / total, 0.0)
    w = np.where(
        ((sample_f >= -0.5) & (sample_f <= in_size - 0.5))[None, :], w, 0.0
    )
    return w.astype(np.float32)  # (in_size, out_size)


def _bicubic_base(x_lr):
    w = _resize_weight_mat(LRS, HR)  # (256, 1024)
    flat = x_lr.reshape(B * C, LRS, LRS)
    t = np.matmul(w.T[None].astype(np.float32), flat)       # (BC, 1024, 256)
    out = np.matmul(t, w[None].astype(np.float32))          # (BC, 1024, 1024)
    return out.reshape(B, C, HR, HR)


# ------------------------------------------------------------- device kernel
def _build_bass():
    import concourse.bacc as bacc
    import concourse.mybir as mybir
    from concourse.tile import TileContext
    from concourse.alu_op_type import AluOpType

    nc = bacc.Bacc(None, target_bir_lowering=False)
    x = nc.dram_tensor("x", [N, 2048], mybir.dt.uint8, kind="ExternalInput")
    y = nc.dram_tensor("y", [N, 1540], mybir.dt.uint8, kind="ExternalOutput")

    KT = N // 128   # 8 contraction tiles
    NT = N // 128   # 8 output-row tiles
    ND = D // 512   # 6 psum tiles per output-row tile

    f8 = mybir.dt.float8e3
    f32 = mybir.dt.float32
    u8 = mybir.dt.uint8

    with TileContext(nc) as tc:
        with (
            tc.tile_pool(name="xp", bufs=1) as xp,
            tc.tile_pool(name="qp", bufs=2) as qp,
            tc.tile_pool(name="otp", bufs=2) as otp,
            tc.tile_pool(name="psp", bufs=1, space="PSUM") as psp,
        ):
            at_sb, hf_sb = [], []
            for k in range(KT):
                xt = xp.tile([128, 2048], u8, name=f"x{k}")
                nc.sync.dma_start(xt[:], x[k * 128 : (k + 1) * 128, :])
                at = xp.tile([128, N], f8, name=f"a{k}")
                ht = xp.tile([128, D], f8, name=f"h{k}")
                # walrus rejects fusing bitwise with arith ops (and, to be
                # safe, dtype-converting writes from bitwise ops), so unpack
                # is: pure-bitwise u8->u8, then an arith convert to fp8.
                ta = qp.tile([128, 512], u8, name="ta", tag="ta")
                tb = qp.tile([128, 512], u8, name="tb", tag="tb")
                tl = qp.tile([128, 1536], u8, name="tl", tag="tl")
                th = qp.tile([128, 1536], u8, name="th", tag="th")
                # attn nibbles: unsigned 0..15
                nc.vector.tensor_scalar(
                    ta[:], xt[:, 0:512], 15, None, AluOpType.bitwise_and
                )
                nc.vector.tensor_copy(at[:, 0:512], ta[:])
                nc.vector.tensor_scalar(
                    tb[:], xt[:, 0:512], 4, None, AluOpType.logical_shift_right
                )
                nc.vector.tensor_copy(at[:, 512:1024], tb[:])
                # hf nibbles: offset-8 signed
                nc.vector.tensor_scalar(
                    tl[:], xt[:, 512:2048], 15, None, AluOpType.bitwise_and
                )
                nc.vector.tensor_scalar_sub(ht[:, 0:1536], tl[:], 8)
                nc.vector.tensor_scalar(
                    th[:], xt[:, 512:2048], 4, None,
                    AluOpType.logical_shift_right,
                )
                nc.vector.tensor_scalar_sub(ht[:, 1536:3072], th[:], 8)
                at_sb.append(at)
                hf_sb.append(ht)

            for n in range(NT):
                ncols = slice(n * 128, (n + 1) * 128)
                ps = [
                    psp.tile([128, 512], f32, name=f"ps{d}", tag=f"ps{d}")
                    for d in range(ND)
                ]
                for k in range(KT):
                    for d in range(ND):
                        nc.tensor.matmul(
                            ps[d][:],
                            at_sb[k][:, ncols],
                            hf_sb[k][:, d * 512 : (d + 1) * 512],
                            start=(k == 0),
                            stop=(k == KT - 1),
                        )
                # per-row |max| over all 6 tiles -> quant scale
                m6 = qp.tile([128, ND], f32, name="m6", tag="m6")
                for d in range(ND):
                    nc.vector.tensor_reduce(
                        m6[:, d : d + 1], ps[d][:], mybir.AxisListType.X,
                        AluOpType.max, apply_absolute_value=True,
                    )
                M = qp.tile([128, 1], f32, name="M", tag="M")
                nc.vector.tensor_reduce(
                    M[:], m6[:], mybir.AxisListType.X, AluOpType.max
                )
                nc.vector.tensor_scalar_max(M[:], M[:], 1e-20)
                S = qp.tile([128, 1], f32, name="S", tag="S")
                nc.vector.reciprocal(S[:], M[:])
                nc.vector.tensor_scalar_mul(S[:], S[:], 7.49)

                qt = [
                    qp.tile([128, 512], u8, name=f"q{d}", tag=f"q{d}")
                    for d in range(ND)
                ]
                t1 = [
                    qp.tile([128, 512], f32, name=f"t{d}", tag=f"t{d}")
                    for d in range(ND)
                ]
                for d in range(ND):
                    # HW f32->u8 conversion rounds-to-nearest (CoreSim
                    # truncates!), so offset by 8.0: u8(clamp(ps*S + 8))
                    # = round(ps*S) + 8 on hardware.
                    nc.vector.tensor_scalar(
                        t1[d][:], ps[d][:], S[:], 8.0,
                        AluOpType.mult, AluOpType.add,
                    )
                    nc.vector.tensor_scalar(
                        qt[d][:], t1[d][:], 0.0, 15.49,
                        AluOpType.max, AluOpType.min,
                    )
                ot = otp.tile([128, 1540], u8, name="ot", tag="ot")
                for d in range(3):
                    nc.vector.scalar_tensor_tensor(
                        ot[:, d * 512 : (d + 1) * 512],
                        qt[d + 3][:], 16.0, qt[d][:],
                        AluOpType.mult, AluOpType.add,
                    )
                nc.vector.tensor_copy(
                    ot[:, 1536:1540], M[:].bitcast(u8)
                )
                nc.gpsimd.dma_start(y[n * 128 : (n + 1) * 128, :], ot[:])
    nc.compile()
    return nc


def _get_nc():
    if "nc" not in _CACHE:
        _CACHE["nc"] = _build_bass()
    return _CACHE["nc"]


# ---------------------------------------------------------------- entrypoint
def kernel(x_hr, x_lr_inpainted, attn_map):
    global LAST_RESULTS
    from concourse import bass_utils

    x_hr = np.asarray(x_hr, dtype=np.float32)
    x_lr = np.asarray(x_lr_inpainted, dtype=np.float32)
    attn = np.asarray(attn_map, dtype=np.float32)

    # high-frequency residual -> patch layout [m=(i,j), d=(c,ph,pw)]
    hp = x_hr - _blur(x_hr)
    hfm = (
        hp.reshape(B, C, HR // P, P, HR // P, P)
        .transpose(0, 2, 4, 1, 3, 5)
        .reshape(B, N, D)
    )
    attnT = np.ascontiguousarray(attn[:, 0].transpose(0, 2, 1))

    # int4 quantization (per batch element = per core)
    sa = 15.0 / np.maximum(attnT.reshape(B, -1).max(axis=1), 1e-30)   # (B,)
    sh = 7.49 / np.maximum(
        HF_CLIP_SIGMA * hfm.reshape(B, -1).std(axis=1), 1e-30
    )
    qa = np.clip(
        np.round(attnT * sa[:, None, None]), 0, 15
    ).astype(np.uint8)
    qh = (
        np.clip(np.round(hfm * sh[:, None, None]), -8, 7) + 8
    ).astype(np.uint8)

    xin = np.empty((B, N, 2048), dtype=np.uint8)
    xin[:, :, 0:512] = qa[:, :, 0:512] + (qa[:, :, 512:1024] << 4)
    xin[:, :, 512:2048] = qh[:, :, 0:1536] + (qh[:, :, 1536:3072] << 4)

    nc = _get_nc()
    if not os.environ.get("KERNEL_TRACE"):
        # NTFF profiling hook (antenv.axon_hooks) is absent in this
        # container; a stray BASS_TRACE=1 would crash the run.
        os.environ["BASS_NEVER_TRACE"] = "1"
    in_maps = [{"x": xin[b]} for b in range(N_CORES)]
    res = bass_utils.run_bass_kernel_spmd(
        nc, in_maps, core_ids=list(range(N_CORES)),
        trace=bool(os.environ.get("KERNEL_TRACE")),
    )
    LAST_RESULTS = res
    _CACHE["in_maps"] = in_maps

    yout = np.stack([np.asarray(res.results[b]["y"]) for b in range(N_CORES)])
    rowmax = yout[:, :, 1536:1540].copy().view(np.float32)[:, :, 0]  # (B, N)
    step = rowmax / 7.49                                    # device units
    lo = (yout[:, :, 0:1536] & 15).astype(np.float32) - 8.0
    hi = (yout[:, :, 0:1536] >> 4).astype(np.float32) - 8.0
    rec = np.empty((B, N, D), dtype=np.float32)
    rec[:, :, 0:1536] = lo
    rec[:, :, 1536:3072] = hi
    rec *= (step / (sa * sh)[:, None])[:, :, None]

    rec_img = (
        rec.reshape(B, HR // P, HR // P, C, P, P)
        .transpose(0, 3, 1, 4, 2, 5)
        .reshape(B, C, HR, HR)
    )
    base = _bicubic_base(x_lr)
    return (base + rec_img).astype(np.float32)


def time_device(n=5):
    """Best-of-n wall time of the device invocation (post-compile)."""
    import time as _time

    from concourse import bass_utils

    nc = _get_nc()
    in_maps = _CACHE["in_maps"]
    best = float("inf")
    for _ in range(n):
        t0 = _time.time()
        bass_utils.run_bass_kernel_spmd(
            nc, in_maps, core_ids=list(range(N_CORES))
        )
        best = min(best, _time.time() - t0)
    return best
